# revision 2
# baseline (speedup 1.0000x reference)
"""Flat paged-attention (vLLM flat_pa, GQA, const-normalized softmax) on 8 TRN2 cores.

Sharding: data-parallel over decode sequences. Core c owns sequences
[8c, 8c+8) = 256 fetched blocks. The host gathers each core's K/V blocks
from the caches (the block_list indirection), converts them to bf16, and
lays them out so the device kernel is a dense stream:

  kt[h, d, (s,n,p)]  -- K gathered + transposed so head-dim is the SBUF
                        partition axis (QK^T contracts over d), bf16
  vt[h, p, (s,n,d)]  -- V gathered, pos on partitions (PV contracts over pos),
                        with a ones column appended per block, bf16
  qt[d, (h,s,q)]     -- queries, scale baked in, bf16
  biast[p, (s,n,q)]  -- block bias with -CONST_VAL baked in, repeated over q, f32

Per (head, seq): 32 K-stationary bf16 matmuls give scores^T [pos, 4q] in PSUM
(fp32), DVE adds bias, ACT exps to bf16, then 32 accumulating PV matmuls +
the ones column give output [4, 128] and the group softmax denominator.
Division by the per-sequence denominator happens once at the end (valid
because the const-normalized softmax denominator is shared by all blocks
of a sequence).

bf16 halves HBM traffic (the memory roofline) and runs the PE at 1
cycle/row with fast-weight-load instead of fp32's 4 cycles/row.
"""

import sys

sys.path.insert(0, "/opt/trn_rl_repo")

import numpy as np
import ml_dtypes

BF16 = ml_dtypes.bfloat16

B = 64
BPS = 32           # blocks per sequence
BS = 128           # block size (tokens)
KVH = 8
QPK = 4            # q heads per kv head
HD = 128
NCORES = 8
SPC = 8            # sequences per core
BPC = SPC * BPS    # 256 blocks per core
CSEQ = 4           # sequences per DMA chunk
CONST_VAL = 10.0
SCALE = 1.0 / np.sqrt(HD)
NEG = -30000.0

_NC_CACHE = {}


def build_nc(reps=1):
    """Build + compile the per-core Bass program. reps>1 wraps the body in a
    dynamic For_i loop (used only for timing)."""
    key = reps
    if key in _NC_CACHE:
        return _NC_CACHE[key]
    from concourse import bacc, mybir
    import concourse.tile as tile

    f32 = mybir.dt.float32
    bf16 = mybir.dt.bfloat16
    nc = bacc.Bacc("TRN2", target_bir_lowering=False, debug=False, num_devices=NCORES)

    kt = nc.dram_tensor("kt", [KVH, HD, BPC * BS], bf16, kind="ExternalInput")
    vt = nc.dram_tensor("vt", [KVH, BS, BPC * (HD + 1)], bf16, kind="ExternalInput")
    qt = nc.dram_tensor("qt", [HD, KVH * SPC * QPK], bf16, kind="ExternalInput")
    biast = nc.dram_tensor("biast", [BS, SPC * BPS * QPK], f32, kind="ExternalInput")
    out = nc.dram_tensor("out", [QPK, KVH * SPC * HD], f32, kind="ExternalOutput")

    CH = CSEQ * BPS * BS        # K cols per chunk
    CHV = CSEQ * BPS * (HD + 1)  # V cols per chunk incl. ones column per block

    with tile.TileContext(nc) as tc:
        from contextlib import ExitStack

        with ExitStack() as ctx:
            cpool = ctx.enter_context(tc.tile_pool(name="const", bufs=1))
            kpool = ctx.enter_context(tc.tile_pool(name="k", bufs=2))
            vpool = ctx.enter_context(tc.tile_pool(name="v", bufs=2))
            ppool = ctx.enter_context(tc.tile_pool(name="p", bufs=3))
            rpool = ctx.enter_context(tc.tile_pool(name="r", bufs=2))
            opool = ctx.enter_context(tc.tile_pool(name="osb", bufs=1))
            qkps = ctx.enter_context(tc.tile_pool(name="qkps", bufs=3, space="PSUM"))
            ops = ctx.enter_context(tc.tile_pool(name="ops", bufs=3, space="PSUM"))

            qt_sb = cpool.tile([HD, KVH * SPC * QPK], bf16)
            nc.sync.dma_start(out=qt_sb[:], in_=qt[:])
            bias_sb = cpool.tile([BS, SPC * BPS * QPK], f32)
            nc.sync.dma_start(out=bias_sb[:], in_=biast[:])
            out_sb = opool.tile([QPK, KVH * SPC * HD], f32)

            def body():
                for h in range(KVH):
                    for sp in range(SPC // CSEQ):
                        kch = kpool.tile([HD, CH], bf16)
                        nc.sync.dma_start(
                            out=kch[:], in_=kt[h, :, sp * CH:(sp + 1) * CH]
                        )
                        vch = vpool.tile([BS, CHV], bf16)
                        nc.sync.dma_start(
                            out=vch[:], in_=vt[h, :, sp * CHV:(sp + 1) * CHV]
                        )
                        for sl in range(CSEQ):
                            s = sp * CSEQ + sl
                            qk = qkps.tile([BS, BPS * QPK], f32)
                            qcol = (h * SPC + s) * QPK
                            for nl in range(BPS):
                                nc.tensor.matmul(
                                    out=qk[:, nl * QPK:(nl + 1) * QPK],
                                    lhsT=kch[:, (sl * BPS + nl) * BS:(sl * BPS + nl + 1) * BS],
                                    rhs=qt_sb[:, qcol:qcol + QPK],
                                    start=True,
                                    stop=True,
                                )
                            pb = ppool.tile([BS, BPS * QPK], f32, tag="pb")
                            nc.vector.tensor_add(
                                out=pb[:],
                                in0=qk[:],
                                in1=bias_sb[:, s * BPS * QPK:(s + 1) * BPS * QPK],
                            )
                            pe = ppool.tile([BS, BPS * QPK], bf16, tag="pe")
                            nc.scalar.activation(
                                pe[:], pb[:], mybir.ActivationFunctionType.Exp
                            )
                            o_ps = ops.tile([QPK, HD + 1], f32)
                            for nl in range(BPS):
                                b = sl * BPS + nl
                                nc.tensor.matmul(
                                    out=o_ps[:],
                                    lhsT=pe[:, nl * QPK:(nl + 1) * QPK],
                                    rhs=vch[:, b * (HD + 1):(b + 1) * (HD + 1)],
                                    start=(nl == 0),
                                    stop=(nl == BPS - 1),
                                )
                            rec = rpool.tile([QPK, 1], f32)
                            nc.vector.reciprocal(rec[:], o_ps[:, HD:HD + 1])
                            nc.vector.tensor_scalar_mul(
                                out_sb[:, (h * SPC + s) * HD:(h * SPC + s + 1) * HD],
                                o_ps[:, 0:HD],
                                rec[:],
                            )
                nc.sync.dma_start(out=out[:], in_=out_sb[:])

            if reps == 1:
                body()
            else:
                with tc.For_i(0, reps, 1):
                    body()

    nc.compile()
    _NC_CACHE[key] = nc
    return nc


def prep_inputs(query, key_cache, value_cache, block_list, block_mapping,
                block_bias, block_groups):
    """Host-side shard + gather + bf16 layout. Returns per-core in_maps."""
    query = np.asarray(query, dtype=np.float32)
    key_cache = np.asarray(key_cache, dtype=np.float32)
    value_cache = np.asarray(value_cache, dtype=np.float32)
    block_list = np.asarray(block_list)
    block_bias = np.asarray(block_bias, dtype=np.float32)
    block_groups = np.asarray(block_groups)

    # per-sequence fetched-block rows (pad to BPS with masked dummies)
    seq_rows = np.zeros((B, BPS), dtype=np.int64)
    pad_mask = np.zeros((B, BPS), dtype=bool)
    for s in range(B):
        rows = np.flatnonzero(block_groups == s)
        assert len(rows) <= BPS, f"sequence {s} has {len(rows)} > {BPS} blocks"
        seq_rows[s, :len(rows)] = rows
        pad_mask[s, len(rows):] = True

    qs = (query.reshape(B, KVH, QPK, HD) * SCALE)  # (s, h, q, d)

    in_maps = []
    for c in range(NCORES):
        rows = seq_rows[c * SPC:(c + 1) * SPC].reshape(-1)          # [256]
        pmask = pad_mask[c * SPC:(c + 1) * SPC].reshape(-1)         # [256]
        bl = block_list[rows].astype(np.int64)
        gk = key_cache[bl]                                           # [256,p,h,d]
        gv = value_cache[bl]
        kt_c = np.ascontiguousarray(
            gk.transpose(2, 3, 0, 1).astype(BF16)).reshape(KVH, HD, -1)
        gv = np.concatenate(
            [gv, np.ones((BPC, BS, KVH, 1), dtype=np.float32)], axis=3)
        vt_c = np.ascontiguousarray(
            gv.transpose(2, 1, 0, 3).astype(BF16)).reshape(KVH, BS, -1)
        # queries for this core: (d, h, s, q)
        qt_c = np.ascontiguousarray(
            qs[c * SPC:(c + 1) * SPC].transpose(3, 1, 0, 2).astype(BF16)
        ).reshape(HD, -1)
        # bias - CONST_VAL, padded blocks fully masked, repeated over q: (p,(s,n,q))
        bia = block_bias[rows] - CONST_VAL                           # [256, p]
        bia[pmask] = NEG - CONST_VAL
        biast_c = np.ascontiguousarray(
            np.repeat(bia.T[:, :, None], QPK, axis=2)
        ).reshape(BS, -1)
        in_maps.append({
            "kt": kt_c, "vt": vt_c,
            "qt": qt_c,
            "biast": biast_c.astype(np.float32),
        })
    return in_maps


def assemble_output(results):
    out = np.zeros((B, KVH * QPK, HD), dtype=np.float32)
    for c in range(NCORES):
        o = results[c]["out"].reshape(QPK, KVH, SPC, HD)  # (q,h,s,d)
        out[c * SPC:(c + 1) * SPC] = o.transpose(2, 1, 0, 3).reshape(SPC, KVH * QPK, HD)
    return out


def kernel(query, key_cache, value_cache, block_list, block_mapping,
           block_bias, block_groups):
    from concourse.bass_utils import run_bass_kernel_spmd

    nc = build_nc(reps=1)
    in_maps = prep_inputs(query, key_cache, value_cache, block_list,
                          block_mapping, block_bias, block_groups)
    res = run_bass_kernel_spmd(nc, in_maps, core_ids=list(range(NCORES)))
    return assemble_output(res.results)


# revision 3
# speedup vs baseline: 2.8593x; 2.8593x over previous
"""Flat paged-attention (vLLM flat_pa, GQA, const-normalized softmax) on 8 TRN2 cores.

Sharding: data-parallel over decode sequences. Core c owns sequences
[8c, 8c+8) = 256 fetched blocks. The host gathers each core's K/V blocks
from the caches (the block_list indirection) and lays them out so the
device kernel is a dense stream:

  kt[h, d, (s,n,p)]  -- K gathered + transposed so head-dim is the SBUF
                        partition axis (QK^T contracts over d), fp16
  vt[h, p, (s,n,d)]  -- V gathered, pos on partitions (PV contracts over pos),
                        int8 (x24 scale), with a 24-valued column appended per
                        block whose PV dot product gives 24*sum(attn) -- the
                        softmax denominator with the same x24 scale, so the
                        quantization scale cancels exactly in the final divide
  qt[d, (h,s,q)]     -- queries, scale baked in, fp16
  biast[p, (s,n,q)]  -- block bias with -CONST_VAL baked in, repeated over q, f32

The device kernel streams K (fp16) and V (int8) with one DMA pair per
2 sequences; V is upcast int8->bf16 on-chip, alternating per sequence
between the DVE and ACT engines (both otherwise mostly idle; keeping the
pieces small avoids head-of-line blocking in their strict-FIFO queues).

Per (head, seq): 32 K-stationary fp16 matmuls give scores^T [pos, 4q] in PSUM
(fp32), DVE adds bias, ACT exps to bf16, then 32 accumulating PV matmuls
give output [4, 128] and the group softmax denominator. Division by the
per-sequence denominator happens once at the end (valid because the
const-normalized softmax denominator is shared by all blocks of a sequence).

16-bit K + 8-bit V cut HBM traffic (the roofline) to 96 MiB/core and run
the PE at 1 cycle/row with fast-weight-load.
"""

import sys

sys.path.insert(0, "/opt/trn_rl_repo")

import numpy as np
import ml_dtypes

B = 64
BPS = 32           # blocks per sequence
BS = 128           # block size (tokens)
KVH = 8
QPK = 4            # q heads per kv head
HD = 128
NCORES = 8
SPC = 8            # sequences per core
BPC = SPC * BPS    # 256 blocks per core
CSEQ = 2           # sequences per DMA chunk
VQ = 24.0          # V int8 quantization scale (cancels in the final divide)
CONST_VAL = 10.0
SCALE = 1.0 / np.sqrt(HD)
NEG = -30000.0

_NC_CACHE = {}


def build_nc(reps=1):
    """Build + compile the per-core Bass program. reps>1 wraps the body in a
    dynamic For_i loop (used only for timing)."""
    key = reps
    if key in _NC_CACHE:
        return _NC_CACHE[key]
    from concourse import bacc, mybir
    import concourse.tile as tile
    from contextlib import ExitStack

    f32 = mybir.dt.float32
    bf16 = mybir.dt.bfloat16
    fp16 = mybir.dt.float16
    i8 = mybir.dt.int8
    nc = bacc.Bacc("TRN2", target_bir_lowering=False, debug=False, num_devices=NCORES)

    kt = nc.dram_tensor("kt", [KVH, HD, BPC * BS], fp16, kind="ExternalInput")
    vt = nc.dram_tensor("vt", [KVH, BS, BPC * (HD + 1)], i8, kind="ExternalInput")
    qt = nc.dram_tensor("qt", [HD, KVH * SPC * QPK], fp16, kind="ExternalInput")
    biast = nc.dram_tensor("biast", [BS, SPC * BPS * QPK], f32, kind="ExternalInput")
    out = nc.dram_tensor("out", [QPK, KVH * SPC * HD], f32, kind="ExternalOutput")

    CHK = CSEQ * BPS * BS         # K cols per chunk
    CHV = CSEQ * BPS * (HD + 1)   # V cols per chunk
    SCHV = BPS * (HD + 1)         # V cols per sequence

    with tile.TileContext(nc) as tc:
        with ExitStack() as ctx:
            cpool = ctx.enter_context(tc.tile_pool(name="const", bufs=1))
            kpool = ctx.enter_context(tc.tile_pool(name="k", bufs=3))
            v8pool = ctx.enter_context(tc.tile_pool(name="v8", bufs=3))
            vpool = ctx.enter_context(tc.tile_pool(name="v", bufs=3))
            ppool = ctx.enter_context(tc.tile_pool(name="p", bufs=3))
            rpool = ctx.enter_context(tc.tile_pool(name="r", bufs=2))
            opool = ctx.enter_context(tc.tile_pool(name="osb", bufs=1))
            qkps = ctx.enter_context(tc.tile_pool(name="qkps", bufs=3, space="PSUM"))
            ops = ctx.enter_context(tc.tile_pool(name="ops", bufs=3, space="PSUM"))

            qt_sb = cpool.tile([HD, KVH * SPC * QPK], fp16)
            nc.sync.dma_start(out=qt_sb[:], in_=qt[:])
            bias_sb = cpool.tile([BS, SPC * BPS * QPK], f32)
            nc.sync.dma_start(out=bias_sb[:], in_=biast[:])
            out_sb = opool.tile([QPK, KVH * SPC * HD], f32)

            def body():
                for h in range(KVH):
                    for sp in range(SPC // CSEQ):
                        kch = kpool.tile([HD, CHK], fp16)
                        nc.sync.dma_start(
                            out=kch[:], in_=kt[h, :, sp * CHK:(sp + 1) * CHK])
                        v8 = v8pool.tile([BS, CHV], i8)
                        nc.sync.dma_start(
                            out=v8[:], in_=vt[h, :, sp * CHV:(sp + 1) * CHV])
                        vch = vpool.tile([BS, CHV], bf16)
                        for sl in range(CSEQ):
                            s = sp * CSEQ + sl
                            if (h * SPC + s) % 2 == 0:
                                nc.vector.tensor_copy(
                                    out=vch[:, sl * SCHV:(sl + 1) * SCHV],
                                    in_=v8[:, sl * SCHV:(sl + 1) * SCHV])
                            else:
                                nc.scalar.activation(
                                    vch[:, sl * SCHV:(sl + 1) * SCHV],
                                    v8[:, sl * SCHV:(sl + 1) * SCHV],
                                    mybir.ActivationFunctionType.Copy)
                        for sl in range(CSEQ):
                            s = sp * CSEQ + sl
                            qk = qkps.tile([BS, BPS * QPK], f32)
                            qcol = (h * SPC + s) * QPK
                            for nl in range(BPS):
                                nc.tensor.matmul(
                                    out=qk[:, nl * QPK:(nl + 1) * QPK],
                                    lhsT=kch[:, (sl * BPS + nl) * BS:(sl * BPS + nl + 1) * BS],
                                    rhs=qt_sb[:, qcol:qcol + QPK],
                                    start=True, stop=True)
                            pb = ppool.tile([BS, BPS * QPK], f32, tag="pb")
                            nc.vector.tensor_add(
                                out=pb[:], in0=qk[:],
                                in1=bias_sb[:, s * BPS * QPK:(s + 1) * BPS * QPK])
                            pe = ppool.tile([BS, BPS * QPK], bf16, tag="pe")
                            nc.scalar.activation(
                                pe[:], pb[:], mybir.ActivationFunctionType.Exp)
                            o_ps = ops.tile([QPK, HD + 1], f32)
                            for nl in range(BPS):
                                b = sl * BPS + nl
                                nc.tensor.matmul(
                                    out=o_ps[:],
                                    lhsT=pe[:, nl * QPK:(nl + 1) * QPK],
                                    rhs=vch[:, b * (HD + 1):(b + 1) * (HD + 1)],
                                    start=(nl == 0), stop=(nl == BPS - 1))
                            rec = rpool.tile([QPK, 1], f32)
                            nc.vector.reciprocal(rec[:], o_ps[:, HD:HD + 1])
                            nc.vector.tensor_scalar_mul(
                                out_sb[:, (h * SPC + s) * HD:(h * SPC + s + 1) * HD],
                                o_ps[:, 0:HD], rec[:])
                nc.sync.dma_start(out=out[:], in_=out_sb[:])

            if reps == 1:
                body()
            else:
                with tc.For_i(0, reps, 1):
                    body()

    nc.compile()
    _NC_CACHE[key] = nc
    return nc


def prep_inputs(query, key_cache, value_cache, block_list, block_mapping,
                block_bias, block_groups):
    """Host-side shard + gather + quantized layout. Returns per-core in_maps."""
    query = np.asarray(query, dtype=np.float32)
    key_cache = np.asarray(key_cache, dtype=np.float32)
    value_cache = np.asarray(value_cache, dtype=np.float32)
    block_list = np.asarray(block_list)
    block_bias = np.asarray(block_bias, dtype=np.float32)
    block_groups = np.asarray(block_groups)

    # per-sequence fetched-block rows (pad to BPS with masked dummies)
    seq_rows = np.zeros((B, BPS), dtype=np.int64)
    pad_mask = np.zeros((B, BPS), dtype=bool)
    for s in range(B):
        rows = np.flatnonzero(block_groups == s)
        assert len(rows) <= BPS, f"sequence {s} has {len(rows)} > {BPS} blocks"
        seq_rows[s, :len(rows)] = rows
        pad_mask[s, len(rows):] = True

    qs = (query.reshape(B, KVH, QPK, HD) * SCALE)  # (s, h, q, d)

    in_maps = []
    for c in range(NCORES):
        rows = seq_rows[c * SPC:(c + 1) * SPC].reshape(-1)          # [256]
        pmask = pad_mask[c * SPC:(c + 1) * SPC].reshape(-1)         # [256]
        bl = block_list[rows].astype(np.int64)
        gk = key_cache[bl]                                           # [256,p,h,d]
        gv = value_cache[bl]
        kt_c = np.ascontiguousarray(
            gk.transpose(2, 3, 0, 1).astype(np.float16)).reshape(KVH, HD, -1)
        gv = np.concatenate(
            [gv, np.ones((BPC, BS, KVH, 1), dtype=np.float32)], axis=3)
        vt_c = np.clip(np.rint(
            np.ascontiguousarray(gv.transpose(2, 1, 0, 3)) * VQ),
            -127, 127).astype(np.int8).reshape(KVH, BS, -1)
        # queries for this core: (d, h, s, q)
        qt_c = np.ascontiguousarray(
            qs[c * SPC:(c + 1) * SPC].transpose(3, 1, 0, 2).astype(np.float16)
        ).reshape(HD, -1)
        # bias - CONST_VAL, padded blocks fully masked, repeated over q: (p,(s,n,q))
        bia = block_bias[rows] - CONST_VAL                           # [256, p]
        bia[pmask] = NEG - CONST_VAL
        biast_c = np.ascontiguousarray(
            np.repeat(bia.T[:, :, None], QPK, axis=2)
        ).reshape(BS, -1)
        in_maps.append({
            "kt": kt_c, "vt": vt_c,
            "qt": qt_c,
            "biast": biast_c.astype(np.float32),
        })
    return in_maps


def assemble_output(results):
    out = np.zeros((B, KVH * QPK, HD), dtype=np.float32)
    for c in range(NCORES):
        o = results[c]["out"].reshape(QPK, KVH, SPC, HD)  # (q,h,s,d)
        out[c * SPC:(c + 1) * SPC] = o.transpose(2, 1, 0, 3).reshape(SPC, KVH * QPK, HD)
    return out


def kernel(query, key_cache, value_cache, block_list, block_mapping,
           block_bias, block_groups):
    from concourse.bass_utils import run_bass_kernel_spmd

    nc = build_nc(reps=1)
    in_maps = prep_inputs(query, key_cache, value_cache, block_list,
                          block_mapping, block_bias, block_groups)
    res = run_bass_kernel_spmd(nc, in_maps, core_ids=list(range(NCORES)))
    return assemble_output(res.results)


# revision 9
# speedup vs baseline: 2.9583x; 1.0346x over previous
"""Flat paged-attention (vLLM flat_pa, GQA, const-normalized softmax) on 8 TRN2 cores.

Sharding: data-parallel over decode sequences. Core c owns sequences
[8c, 8c+8) = 256 fetched blocks. The host gathers each core's K/V blocks
from the caches (the block_list indirection) and lays them out so the
device kernel is a dense stream:

  kt[h, d, (s,n,p)]  -- K gathered + transposed so head-dim is the SBUF
                        partition axis (QK^T contracts over d), fp16
  vt[h, p, (s,n,d)]  -- V gathered, pos on partitions (PV contracts over pos),
                        int8 with a per-block integer scale m_b: block b stores
                        round(v*m_b) and an extra column holding m_b itself.
                        -ln(m_b) is folded into that block's bias, so the PV
                        accumulation yields sum_b (attn_b/m_b)(v*m_b) = attn@v
                        and the extra column yields sum(attn) -- the scales
                        cancel exactly, no extra device work
  qt[d, (h,s,q)]     -- queries, scale baked in, fp16
  biast[p, (s,n,q)]  -- block bias with -CONST_VAL baked in, repeated over q, f32

The device kernel streams K (fp16) and V (int8) with one DMA pair per
2 sequences; V is upcast int8->bf16 on-chip, alternating per sequence
between the DVE and ACT engines (both otherwise mostly idle; keeping the
pieces small avoids head-of-line blocking in their strict-FIFO queues).

Per (head, seq): 32 K-stationary fp16 matmuls give scores^T [pos, 4q] in PSUM
(fp32), DVE adds bias, ACT exps to bf16, then 32 accumulating PV matmuls
give output [4, 128] and the group softmax denominator. Division by the
per-sequence denominator happens once at the end (valid because the
const-normalized softmax denominator is shared by all blocks of a sequence).

16-bit K + 8-bit V cut HBM traffic (the roofline) to 96 MiB/core and run
the PE at 1 cycle/row with fast-weight-load.
"""

import sys

sys.path.insert(0, "/opt/trn_rl_repo")

import numpy as np
import ml_dtypes

B = 64
BPS = 32           # blocks per sequence
BS = 128           # block size (tokens)
KVH = 8
QPK = 4            # q heads per kv head
HD = 128
NCORES = 8
SPC = 8            # sequences per core
BPC = SPC * BPS    # 256 blocks per core
CSEQ = 4           # sequences per DMA chunk
CONST_VAL = 10.0
SCALE = 1.0 / np.sqrt(HD)
NEG = -30000.0

_NC_CACHE = {}


def build_nc(reps=1):
    """Build + compile the per-core Bass program. reps>1 wraps the body in a
    dynamic For_i loop (used only for timing)."""
    key = reps
    if key in _NC_CACHE:
        return _NC_CACHE[key]
    from concourse import bacc, mybir
    import concourse.tile as tile
    from contextlib import ExitStack

    f32 = mybir.dt.float32
    bf16 = mybir.dt.bfloat16
    fp16 = mybir.dt.float16
    i8 = mybir.dt.int8
    nc = bacc.Bacc("TRN2", target_bir_lowering=False, debug=False, num_devices=NCORES)

    kt = nc.dram_tensor("kt", [KVH, HD, BPC * BS], fp16, kind="ExternalInput")
    vt = nc.dram_tensor("vt", [KVH, BS, BPC * (HD + 1)], i8, kind="ExternalInput")
    qt = nc.dram_tensor("qt", [HD, KVH * SPC * QPK], fp16, kind="ExternalInput")
    biast = nc.dram_tensor("biast", [BS, SPC * BPS * QPK], f32, kind="ExternalInput")
    out = nc.dram_tensor("out", [QPK, KVH * SPC * HD], f32, kind="ExternalOutput")

    CHK = CSEQ * BPS * BS         # K cols per chunk
    CHV = CSEQ * BPS * (HD + 1)   # V cols per chunk
    SCHV = BPS * (HD + 1)         # V cols per sequence

    with tile.TileContext(nc) as tc:
        with ExitStack() as ctx:
            cpool = ctx.enter_context(tc.tile_pool(name="const", bufs=1))
            kpool = ctx.enter_context(tc.tile_pool(name="k", bufs=2))
            v8pool = ctx.enter_context(tc.tile_pool(name="v8", bufs=2))
            vpool = ctx.enter_context(tc.tile_pool(name="v", bufs=2))
            ppool = ctx.enter_context(tc.tile_pool(name="p", bufs=3))
            rpool = ctx.enter_context(tc.tile_pool(name="r", bufs=2))
            opool = ctx.enter_context(tc.tile_pool(name="osb", bufs=1))
            qkps = ctx.enter_context(tc.tile_pool(name="qkps", bufs=3, space="PSUM"))
            ops = ctx.enter_context(tc.tile_pool(name="ops", bufs=3, space="PSUM"))

            qt_sb = cpool.tile([HD, KVH * SPC * QPK], fp16)
            nc.sync.dma_start(out=qt_sb[:], in_=qt[:])
            bias_sb = cpool.tile([BS, SPC * BPS * QPK], f32)
            nc.sync.dma_start(out=bias_sb[:], in_=biast[:])
            out_sb = opool.tile([QPK, KVH * SPC * HD], f32)

            def body():
                for h in range(KVH):
                    for sp in range(SPC // CSEQ):
                        kch = kpool.tile([HD, CHK], fp16)
                        nc.sync.dma_start(
                            out=kch[:], in_=kt[h, :, sp * CHK:(sp + 1) * CHK])
                        v8 = v8pool.tile([BS, CHV], i8)
                        nc.sync.dma_start(
                            out=v8[:], in_=vt[h, :, sp * CHV:(sp + 1) * CHV])
                        vch = vpool.tile([BS, CHV], bf16)
                        for sl in range(CSEQ):
                            # upcast int8->bf16 split between DVE (first ~half)
                            # and ACT (rest): keeps both under the DMA period
                            # and the strict-FIFO queue entries small
                            lo = sl * SCHV
                            mid = lo + (SCHV * 5 // 9)
                            hi = (sl + 1) * SCHV
                            nc.vector.tensor_copy(
                                out=vch[:, lo:mid], in_=v8[:, lo:mid])
                            nc.scalar.activation(
                                vch[:, mid:hi], v8[:, mid:hi],
                                mybir.ActivationFunctionType.Copy)
                        for sl in range(CSEQ):
                            s = sp * CSEQ + sl
                            qk = qkps.tile([BS, BPS * QPK], f32)
                            qcol = (h * SPC + s) * QPK
                            for nl in range(BPS):
                                nc.tensor.matmul(
                                    out=qk[:, nl * QPK:(nl + 1) * QPK],
                                    lhsT=kch[:, (sl * BPS + nl) * BS:(sl * BPS + nl + 1) * BS],
                                    rhs=qt_sb[:, qcol:qcol + QPK],
                                    start=True, stop=True)
                            pb = ppool.tile([BS, BPS * QPK], f32, tag="pb")
                            nc.vector.tensor_add(
                                out=pb[:], in0=qk[:],
                                in1=bias_sb[:, s * BPS * QPK:(s + 1) * BPS * QPK])
                            pe = ppool.tile([BS, BPS * QPK], bf16, tag="pe")
                            nc.scalar.activation(
                                pe[:], pb[:], mybir.ActivationFunctionType.Exp)
                            o_ps = ops.tile([QPK, HD + 1], f32)
                            for nl in range(BPS):
                                b = sl * BPS + nl
                                nc.tensor.matmul(
                                    out=o_ps[:],
                                    lhsT=pe[:, nl * QPK:(nl + 1) * QPK],
                                    rhs=vch[:, b * (HD + 1):(b + 1) * (HD + 1)],
                                    start=(nl == 0), stop=(nl == BPS - 1))
                            rec = rpool.tile([QPK, 1], f32)
                            nc.vector.reciprocal(rec[:], o_ps[:, HD:HD + 1])
                            nc.vector.tensor_scalar_mul(
                                out_sb[:, (h * SPC + s) * HD:(h * SPC + s + 1) * HD],
                                o_ps[:, 0:HD], rec[:])
                nc.sync.dma_start(out=out[:], in_=out_sb[:])

            if reps == 1:
                body()
            else:
                with tc.For_i(0, reps, 1):
                    body()

    nc.compile()
    _NC_CACHE[key] = nc
    return nc


def prep_inputs(query, key_cache, value_cache, block_list, block_mapping,
                block_bias, block_groups):
    """Host-side shard + gather + quantized layout. Returns per-core in_maps."""
    query = np.asarray(query, dtype=np.float32)
    key_cache = np.asarray(key_cache, dtype=np.float32)
    value_cache = np.asarray(value_cache, dtype=np.float32)
    block_list = np.asarray(block_list)
    block_bias = np.asarray(block_bias, dtype=np.float32)
    block_groups = np.asarray(block_groups)

    # per-sequence fetched-block rows (pad to BPS with masked dummies)
    seq_rows = np.zeros((B, BPS), dtype=np.int64)
    pad_mask = np.zeros((B, BPS), dtype=bool)
    for s in range(B):
        rows = np.flatnonzero(block_groups == s)
        assert len(rows) <= BPS, f"sequence {s} has {len(rows)} > {BPS} blocks"
        seq_rows[s, :len(rows)] = rows
        pad_mask[s, len(rows):] = True

    qs = (query.reshape(B, KVH, QPK, HD) * SCALE)  # (s, h, q, d)

    in_maps = []
    for c in range(NCORES):
        rows = seq_rows[c * SPC:(c + 1) * SPC].reshape(-1)          # [256]
        pmask = pad_mask[c * SPC:(c + 1) * SPC].reshape(-1)         # [256]
        bl = block_list[rows].astype(np.int64)
        gk = key_cache[bl]                                           # [256,p,h,d]
        gv = value_cache[bl]
        kt_c = np.ascontiguousarray(
            gk.transpose(2, 3, 0, 1).astype(np.float16)).reshape(KVH, HD, -1)
        # per-block integer scale m_b = floor(127/max|v|) over all heads of the
        # block, clamped to [1, 127]; block b stores round(v*m_b) plus a
        # column holding m_b itself
        bmax = np.abs(gv).max(axis=(1, 2, 3))                        # [256]
        m_b = np.clip(np.floor(127.0 / np.maximum(bmax, 1e-6)),
                      1.0, 127.0).astype(np.float32)                 # [256]
        gq = np.clip(np.rint(gv * m_b[:, None, None, None]), -127, 127)
        gq = np.concatenate(
            [gq, np.broadcast_to(m_b[:, None, None, None], (BPC, BS, KVH, 1))],
            axis=3)
        vt_c = np.ascontiguousarray(
            gq.transpose(2, 1, 0, 3)).astype(np.int8).reshape(KVH, BS, -1)
        # queries for this core: (d, h, s, q)
        qt_c = np.ascontiguousarray(
            qs[c * SPC:(c + 1) * SPC].transpose(3, 1, 0, 2).astype(np.float16)
        ).reshape(HD, -1)
        # bias - CONST_VAL - ln(m_b), padded blocks fully masked; the ln(m_b)
        # term divides each block's exp weights by m_b, which the m_b stored
        # in V's extra column multiplies back -- per-block scales cancel.
        bia = block_bias[rows] - CONST_VAL - np.log(m_b)[:, None]    # [256, p]
        bia[pmask] = NEG - CONST_VAL
        biast_c = np.ascontiguousarray(
            np.repeat(bia.T[:, :, None], QPK, axis=2)
        ).reshape(BS, -1)
        in_maps.append({
            "kt": kt_c, "vt": vt_c,
            "qt": qt_c,
            "biast": biast_c.astype(np.float32),
        })
    return in_maps


def assemble_output(results):
    out = np.zeros((B, KVH * QPK, HD), dtype=np.float32)
    for c in range(NCORES):
        o = results[c]["out"].reshape(QPK, KVH, SPC, HD)  # (q,h,s,d)
        out[c * SPC:(c + 1) * SPC] = o.transpose(2, 1, 0, 3).reshape(SPC, KVH * QPK, HD)
    return out


def kernel(query, key_cache, value_cache, block_list, block_mapping,
           block_bias, block_groups):
    from concourse.bass_utils import run_bass_kernel_spmd

    nc = build_nc(reps=1)
    in_maps = prep_inputs(query, key_cache, value_cache, block_list,
                          block_mapping, block_bias, block_groups)
    res = run_bass_kernel_spmd(nc, in_maps, core_ids=list(range(NCORES)))
    return assemble_output(res.results)
